# revision 10
# baseline (speedup 1.0000x reference)
"""Trainium2 Bass kernel for nn_BaselineGAT (LayerNorm + MLP + GATConv).

Strategy (8 NeuronCores, SPMD, host-mediated phase boundary):
  Phase 1 (per core, nodes sharded 6272/core, degree-bucketed order):
    LayerNorm folded into the first matmul (stats via ones-matmul + Square),
    MLP 1488->1024->512 in float32r (full-rate PE), then row-major heads:
    g table rows (bf16), res rows (bf16), a_src/a_dst side tables (f32).
  Host: concat g-table shards -> full bf16 table [50176, 256] (512B rows);
    build per-edge-slot a_src/a_dst streams by static slot indexing.
  Phase 2 (per core, edges sharded by dst, ragged 128-edge blocks per
    128-dst batch, src table split at 32768 for int16 gather indices):
    dma_gather of g rows (one edge per partition), w = exp(leaky_relu(
    a_src+a_dst)) on DVE/ACT, msg = [g*w | w] bf16, one-hot S = (dcol==iota),
    PE scatter-reduce out[dst,f] = S^T @ msg accumulated over blocks in PSUM,
    then normalize by the denominator column, elu, + res -> output rows.
    Empty slots use idx 0 with a_src = -4e4 so w == 0 exactly.
"""

import sys

sys.path.insert(0, "/opt/trn_rl_repo")

from dataclasses import dataclass

import numpy as np
import ml_dtypes

import concourse.bass as bass  # noqa: F401
import concourse.mybir as mybir
import concourse.tile as tile
from concourse import bacc
from concourse.bass_utils import run_bass_kernel_spmd
from concourse.library_config import mlp as mlp_lib

P = 128
F32 = mybir.dt.float32
F32R = mybir.dt.float32r
BF16 = mybir.dt.bfloat16
I16 = mybir.dt.int16
AL = mybir.AluOpType
AF = mybir.ActivationFunctionType
NPBF16 = ml_dtypes.bfloat16


@dataclass
class Cfg:
    n_nodes: int = 50000
    n_edges: int = 800000
    d_in: int = 1488
    d_hid: int = 1024
    d_out: int = 512
    C: int = 32
    H: int = 8
    n_cores: int = 8
    node_chunk: int = 384  # phase-1 nodes per chunk
    split_cap: int = 32768  # max rows addressable by int16 gather idx
    gmax: int = 8  # max 128-edge blocks (1024 idxs) per dma_gather

    @property
    def d_head(self):  # H*C
        return self.C * self.H

    @property
    def d_in_pad(self):
        return ((self.d_in + P - 1) // P) * P

    @property
    def rows_per_core(self):
        nb = (self.n_nodes + P - 1) // P
        nb = ((nb + self.n_cores - 1) // self.n_cores) * self.n_cores
        return nb // self.n_cores * P

    @property
    def n_batches(self):
        return self.rows_per_core // P

    @property
    def table_rows(self):
        return self.rows_per_core * self.n_cores

    @property
    def split(self):
        # pass-A half of the table (int16 index limit)
        return min(self.split_cap, self.table_rows)


CFG = Cfg()

_NC_CACHE = {}


# ----------------------------------------------------------------------------
# Phase 1: LayerNorm + MLP + heads
# ----------------------------------------------------------------------------

def build_phase1(cfg: Cfg, repeat: int = 1):
    # repeat>1 builds are timing-only variants (same outputs rewritten)
    key = ("p1", cfg.n_nodes, cfg.node_chunk, repeat)
    if key in _NC_CACHE:
        return _NC_CACHE[key]
    nc = bacc.Bacc("TRN2", target_bir_lowering=False)
    R = cfg.rows_per_core
    NB = cfg.n_batches
    KT1 = cfg.d_in_pad // P         # k-tiles layer 1
    KT2 = cfg.d_hid // P            # k-tiles layer 2
    KT3 = cfg.d_out // P            # k-tiles layer 3
    MT1 = cfg.d_hid // P            # m-tiles layer 1
    MT2 = cfg.d_out // P            # m-tiles layer 2
    NCK = cfg.node_chunk
    chunk_sizes = [NCK] * (R // NCK)
    if R % NCK:
        assert R % NCK % P == 0
        chunk_sizes.append(R % NCK)
    W3 = cfg.d_head + cfg.H         # 264
    DH = cfg.d_head

    xT = nc.dram_tensor("xT", [cfg.d_in_pad, R], F32R, kind="ExternalInput")
    W1p = nc.dram_tensor("W1p", [cfg.d_in_pad, cfg.d_hid], F32R, kind="ExternalInput")
    W2 = nc.dram_tensor("W2", [cfg.d_hid, cfg.d_out], F32R, kind="ExternalInput")
    Wgp = nc.dram_tensor("Wgp", [cfg.d_out, W3], F32R, kind="ExternalInput")
    Wrp = nc.dram_tensor("Wrp", [cfg.d_out, W3], F32R, kind="ExternalInput")
    w1s = nc.dram_tensor("w1s", [8, cfg.d_hid], F32R, kind="ExternalInput")  # row0 = -colsum(W1p)
    onep = nc.dram_tensor("onep", [8, P], F32R, kind="ExternalInput")        # row0 = ones
    ones1 = nc.dram_tensor("ones1", [P, 1], F32R, kind="ExternalInput")
    cvec = nc.dram_tensor("cvec", [P, MT1], F32, kind="ExternalInput")      # b1 + ln_b@W1
    b2v = nc.dram_tensor("b2v", [P, MT2], F32, kind="ExternalInput")
    brpad = nc.dram_tensor("brpad", [P, W3], F32, kind="ExternalInput")

    gtab = nc.dram_tensor("gtab", [R, DH], BF16, kind="ExternalOutput")
    res = nc.dram_tensor("res", [R, DH], BF16, kind="ExternalOutput")
    asrc = nc.dram_tensor("asrc", [R, cfg.H], F32, kind="ExternalOutput")
    adst = nc.dram_tensor("adst", [R, cfg.H], F32, kind="ExternalOutput")

    inv_din = 1.0 / cfg.d_in

    with tile.TileContext(nc) as tc:
        with (
            tc.tile_pool(name="wpool", bufs=1) as wp,
            tc.tile_pool(name="xpool", bufs=2) as xp,
            tc.tile_pool(name="sqpool", bufs=2) as sqp,
            tc.tile_pool(name="hpool", bufs=2) as hp,
            tc.tile_pool(name="epool", bufs=3) as ep,
            tc.tile_pool(name="stat", bufs=1) as stp,
            tc.tile_pool(name="ps_y", bufs=2, space="PSUM") as ps_y,
            tc.tile_pool(name="ps_s", bufs=1, space="PSUM") as ps_s,
            tc.tile_pool(name="ps_o", bufs=3, space="PSUM") as ps_o,
        ):
            w1_sb = wp.tile([P, KT1, cfg.d_hid], F32R)
            nc.sync.dma_start(w1_sb[:], W1p.rearrange("(kt p) m -> p kt m", p=P))
            w2_sb = wp.tile([P, KT2, cfg.d_out], F32R)
            nc.sync.dma_start(w2_sb[:], W2.rearrange("(kt p) m -> p kt m", p=P))
            wg_sb = wp.tile([P, KT3, W3], F32R)
            nc.sync.dma_start(wg_sb[:], Wgp.rearrange("(kt p) m -> p kt m", p=P))
            wr_sb = wp.tile([P, KT3, W3], F32R)
            nc.sync.dma_start(wr_sb[:], Wrp.rearrange("(kt p) m -> p kt m", p=P))
            w1s_sb = wp.tile([8, cfg.d_hid], F32R)
            nc.sync.dma_start(w1s_sb[:], w1s[:])
            onep_sb = wp.tile([8, P], F32R)
            nc.sync.dma_start(onep_sb[:], onep[:])
            ones1_sb = wp.tile([P, 1], F32R)
            nc.sync.dma_start(ones1_sb[:], ones1[:])
            cvec_sb = wp.tile([P, MT1], F32)
            nc.sync.dma_start(cvec_sb[:], cvec[:])
            b2_sb = wp.tile([P, MT2], F32)
            nc.sync.dma_start(b2_sb[:], b2v[:])
            brp_sb = wp.tile([P, W3], F32)
            nc.sync.dma_start(brp_sb[:], brpad[:])
            asrc_acc = wp.tile([P, NB, cfg.H], F32)
            adst_acc = wp.tile([P, NB, cfg.H], F32)

            for _rep in range(repeat):
                ns = 0
                for NC in chunk_sizes:
                    # ---- load xT chunk [P, KT1, NC]
                    xt = xp.tile([P, KT1, NC], F32R, tag="xt")
                    nc.sync.dma_start(
                        xt[:],
                        xT.rearrange("(kt p) n -> p kt n", p=P)[:, :, ns:ns + NC]
                    )
                    # ---- stats: S1 = ones^T @ x ; S2 = ones^T @ x^2
                    s1_ps = ps_s.tile([1, NC], F32, tag="s1")
                    s2_ps = ps_s.tile([1, NC], F32, tag="s2")
                    for kt in range(KT1):
                        nc.tensor.matmul(s1_ps[:], ones1_sb[:], xt[:, kt],
                                         start=(kt == 0), stop=(kt == KT1 - 1))
                    for kt in range(KT1):
                        xsq = sqp.tile([P, NC], F32R, tag="xsq")
                        nc.scalar.activation(xsq[:], xt[:, kt], AF.Square)
                        nc.tensor.matmul(s2_ps[:], ones1_sb[:], xsq[:],
                                         start=(kt == 0), stop=(kt == KT1 - 1))
                    # ---- finalize stats: mu, rstd (on [1, NC])
                    mu_f = stp.tile([8, NC], F32, tag="muf")
                    nc.vector.memset(mu_f[:], 0.0)
                    nc.vector.tensor_scalar_mul(mu_f[0:1, :], s1_ps[:], inv_din)
                    mu_pad = stp.tile([8, NC], F32R, tag="mu")
                    nc.scalar.activation(mu_pad[:], mu_f[:], AF.Identity)
                    musq = stp.tile([1, NC], F32, tag="musq")
                    nc.vector.tensor_tensor(musq[:], mu_f[0:1, :], mu_f[0:1, :],
                                            op=AL.mult)
                    var = stp.tile([1, NC], F32, tag="var")
                    # var = S2/din - mu^2 + eps
                    nc.vector.tensor_scalar(var[:], s2_ps[:], inv_din, None,
                                            op0=AL.mult)
                    nc.vector.tensor_tensor(var[:], var[:], musq[:],
                                            op=AL.subtract)
                    nc.vector.tensor_scalar_add(var[:], var[:], 1e-5)
                    sd = stp.tile([8, NC], F32, tag="sd")
                    nc.scalar.activation(sd[0:1, :], var[:], AF.Sqrt)
                    rstd_f = stp.tile([8, NC], F32, tag="rstdf")
                    nc.vector.memset(rstd_f[:], 0.0)
                    nc.vector.reciprocal(rstd_f[0:1, :], sd[0:1, :])
                    rstd = stp.tile([8, NC], F32R, tag="rstd")
                    nc.scalar.activation(rstd[:], rstd_f[:], AF.Identity)
                    # broadcast rstd to [P, NC] via K=8 matmul
                    rb_ps = ps_s.tile([P, NC], F32, tag="rb")
                    nc.tensor.matmul(rb_ps[:], onep_sb[:], rstd[:],
                                     start=True, stop=True)
                    rstd_b = stp.tile([P, NC], F32, tag="rstdb")
                    nc.vector.tensor_copy(rstd_b[:], rb_ps[:])

                    # ---- layer 1: y = W1p^T x - w1sum (x) mu; h = relu(y*rstd+c)
                    h_sb = hp.tile([P, MT1, NC], F32R, tag="h")
                    for mt in range(MT1):
                        y_ps = ps_y.tile([P, NC], F32, tag="y")
                        for kt in range(KT1):
                            nc.tensor.matmul(
                                y_ps[:], w1_sb[:, kt, mt * P:(mt + 1) * P],
                                xt[:, kt], start=(kt == 0), stop=False)
                        nc.tensor.matmul(
                            y_ps[:], w1s_sb[:, mt * P:(mt + 1) * P], mu_pad[:],
                            start=False, stop=True)
                        tmp = ep.tile([P, NC], F32, tag="l1t")
                        nc.vector.tensor_tensor(tmp[:], y_ps[:], rstd_b[:],
                                                op=AL.mult)
                        nc.scalar.activation(h_sb[:, mt], tmp[:], AF.Relu,
                                             bias=cvec_sb[:, mt:mt + 1])

                    # ---- layer 2: h2 = W2^T h + b2
                    h2_sb = hp.tile([P, MT2, NC], F32R, tag="h2")
                    for mt in range(MT2):
                        y2_ps = ps_y.tile([P, NC], F32, tag="y")
                        for kt in range(KT2):
                            nc.tensor.matmul(
                                y2_ps[:], w2_sb[:, kt, mt * P:(mt + 1) * P],
                                h_sb[:, kt], start=(kt == 0),
                                stop=(kt == KT2 - 1))
                        nc.scalar.activation(h2_sb[:, mt], y2_ps[:], AF.Identity,
                                             bias=b2_sb[:, mt:mt + 1])

                    # ---- layer 3 (row-major): per 128-node subtile
                    for nt in range(NC // P):
                        g_ps = ps_o.tile([P, W3], F32, tag="ops")
                        r_ps = ps_o.tile([P, W3], F32, tag="ops")
                        for kt in range(KT3):
                            nc.tensor.matmul(
                                g_ps[:], h2_sb[:, kt, nt * P:(nt + 1) * P],
                                wg_sb[:, kt], start=(kt == 0),
                                stop=(kt == KT3 - 1))
                        for kt in range(KT3):
                            nc.tensor.matmul(
                                r_ps[:], h2_sb[:, kt, nt * P:(nt + 1) * P],
                                wr_sb[:, kt], start=(kt == 0),
                                stop=(kt == KT3 - 1))
                        r0 = ns + nt * P
                        bidx = r0 // P
                        gt = ep.tile([P, DH], BF16, tag="gt")
                        nc.scalar.activation(gt[:], g_ps[:, :DH], AF.Identity)
                        nc.vector.tensor_copy(asrc_acc[:, bidx], g_ps[:, DH:W3])
                        rt = ep.tile([P, DH], BF16, tag="rt")
                        nc.vector.tensor_tensor(rt[:], r_ps[:, :DH],
                                                brp_sb[:, :DH], op=AL.add)
                        nc.vector.tensor_copy(adst_acc[:, bidx], r_ps[:, DH:W3])
                        nc.sync.dma_start(gtab[r0:r0 + P, :], gt[:])
                        nc.sync.dma_start(res[r0:r0 + P, :], rt[:])
                    ns += NC
            nc.sync.dma_start(asrc.rearrange("(b p) w -> p b w", p=P), asrc_acc[:])
            nc.sync.dma_start(adst.rearrange("(b p) w -> p b w", p=P), adst_acc[:])
    nc.compile()
    _NC_CACHE[key] = nc
    return nc


# ----------------------------------------------------------------------------
# Phase 2: gather + edge softmax + one-hot scatter matmul + finalize
# ----------------------------------------------------------------------------

def build_phase2(cfg: Cfg, nblkA: list, nblkB: list, repeat: int = 1):
    """nblkA/nblkB: per-batch 128-edge block counts for the A half
    (src row < split) and B half of the table; shared across cores."""
    key = ("p2", cfg.n_nodes, tuple(nblkA), tuple(nblkB), repeat)
    if key in _NC_CACHE:
        return _NC_CACHE[key]
    nc = bacc.Bacc("TRN2", target_bir_lowering=False)
    R = cfg.rows_per_core
    NB = cfg.n_batches
    DH = cfg.d_head
    H = cfg.H
    C = cfg.C
    W = DH + H  # msg row: [g*w | w]
    assert len(nblkA) == NB and len(nblkB) == NB
    NBT = int(sum(nblkA) + sum(nblkB))
    NBLKMAX = int(max(a + b for a, b in zip(nblkA, nblkB)))
    GMAX = cfg.gmax

    gtab = nc.dram_tensor("gtab", [cfg.table_rows, DH], BF16, kind="ExternalInput")
    idx = nc.dram_tensor("idx", [P, 8 * NBT], I16, kind="ExternalInput")
    asrcS = nc.dram_tensor("asrcS", [P, NBT, H], BF16, kind="ExternalInput")
    adstS = nc.dram_tensor("adstS", [P, NBT, H], BF16, kind="ExternalInput")
    dcol = nc.dram_tensor("dcol", [P, NBT], BF16, kind="ExternalInput")
    iot = nc.dram_tensor("iot", [P, P], BF16, kind="ExternalInput")
    resi = nc.dram_tensor("resi", [R, DH], BF16, kind="ExternalInput")
    bgb = nc.dram_tensor("bgb", [P, DH], F32, kind="ExternalInput")
    outp = nc.dram_tensor("outp", [R, DH], F32, kind="ExternalOutput")

    tabA = gtab[:cfg.split, :]
    tabB = gtab[cfg.split:, :]

    with tile.TileContext(nc) as tc:
        with (
            tc.tile_pool(name="const", bufs=1) as cp,
            tc.tile_pool(name="gpool", bufs=2) as gp,
            tc.tile_pool(name="spool", bufs=2) as sp,
            tc.tile_pool(name="mpool", bufs=2) as mp,
            tc.tile_pool(name="wk", bufs=3) as wk,
            tc.tile_pool(name="fin", bufs=3) as fin,
            tc.tile_pool(name="pso", bufs=4, space="PSUM") as pso,
        ):
            nc.gpsimd.load_library(mlp_lib)
            idx_sb = cp.tile([P, 8 * NBT], I16)
            nc.sync.dma_start(idx_sb[:], idx[:])
            asrc_sb = cp.tile([P, NBT, H], BF16)
            nc.sync.dma_start(asrc_sb[:], asrcS[:])
            adst_sb = cp.tile([P, NBT, H], BF16)
            nc.sync.dma_start(adst_sb[:], adstS[:])
            dcol_sb = cp.tile([P, NBT], BF16)
            nc.sync.dma_start(dcol_sb[:], dcol[:])
            iot_sb = cp.tile([P, P], BF16)
            nc.sync.dma_start(iot_sb[:], iot[:])
            res_sb = cp.tile([P, NB, DH], BF16)
            nc.sync.dma_start(res_sb[:], resi.rearrange("(b p) w -> p b w", p=P))
            bg_sb = cp.tile([P, DH], F32)
            nc.sync.dma_start(bg_sb[:], bgb[:])

            for _rep in range(repeat):
                t0 = 0
                for b in range(NB):
                    na, nbk = int(nblkA[b]), int(nblkB[b])
                    NBLK = na + nbk
                    assert NBLK >= 1
                    G = gp.tile([P, NBLKMAX, DH], BF16, tag="g")
                    # gathers: A blocks then B blocks (<=GMAX blocks per call)
                    for part, cnt, base, tab in ((0, na, 0, tabA),
                                                 (1, nbk, na, tabB)):
                        for j0 in range(0, cnt, GMAX):
                            kk = min(GMAX, cnt - j0)
                            ni = P * kk
                            tcol = 8 * (t0 + base + j0)
                            nc.gpsimd.dma_gather(
                                G[:, base + j0:base + j0 + kk, :], tab,
                                idx_sb[:, tcol:tcol + 8 * kk], ni, ni, DH,
                            )
                    # w = exp(leaky_relu(a_src + a_dst))
                    ww = wk.tile([P, NBLKMAX, H], F32, tag="ww")
                    nc.vector.tensor_tensor(ww[:, :NBLK],
                                            asrc_sb[:, t0:t0 + NBLK],
                                            adst_sb[:, t0:t0 + NBLK], op=AL.add)
                    nc.vector.scalar_tensor_tensor(
                        ww[:, :NBLK], ww[:, :NBLK], 0.2, ww[:, :NBLK],
                        op0=AL.mult, op1=AL.max)
                    wb = wk.tile([P, NBLKMAX, H], BF16, tag="wb")
                    nc.scalar.activation(wb[:, :NBLK], ww[:, :NBLK], AF.Exp)
                    # msg = [g * w | w]
                    M = mp.tile([P, NBLKMAX, W], BF16, tag="m")
                    nc.vector.tensor_tensor(
                        M[:, :NBLK, :DH].rearrange("p k (h c) -> p k h c", c=C),
                        G[:, :NBLK].rearrange("p k (h c) -> p k h c", c=C),
                        wb[:, :NBLK].unsqueeze(3).to_broadcast([P, NBLK, H, C]),
                        op=AL.mult)
                    nc.vector.tensor_copy(M[:, :NBLK, DH:W], wb[:, :NBLK])
                    # one-hot S[e, d] = (dcol[e] == d)
                    S = sp.tile([P, NBLKMAX, P], BF16, tag="s")
                    nc.vector.tensor_tensor(
                        S[:, :NBLK],
                        dcol_sb[:, t0:t0 + NBLK].unsqueeze(2)
                            .to_broadcast([P, NBLK, P]),
                        iot_sb[:].unsqueeze(1).to_broadcast([P, NBLK, P]),
                        op=AL.is_equal)
                    # scatter-reduce: out[d, f] = sum_e S[e,d] * msg[e,f]
                    o_ps = pso.tile([P, W], F32, tag="o")
                    for j in range(NBLK):
                        nc.tensor.matmul(o_ps[:], S[:, j], M[:, j],
                                         start=(j == 0), stop=(j == NBLK - 1))
                    # normalize, elu, + res
                    rec = fin.tile([P, H], F32, tag="rec")
                    nc.vector.tensor_scalar_add(rec[:], o_ps[:, DH:W], 1e-16)
                    nc.vector.reciprocal(rec[:], rec[:])
                    z = fin.tile([P, DH], F32, tag="z")
                    nc.vector.tensor_tensor(
                        z[:].rearrange("p (h c) -> p h c", c=C),
                        o_ps[:, :DH].rearrange("p (h c) -> p h c", c=C),
                        rec[:].unsqueeze(2).to_broadcast([P, H, C]),
                        op=AL.mult)
                    v = fin.tile([P, DH], BF16, tag="v")
                    nc.vector.tensor_tensor(v[:], z[:], bg_sb[:], op=AL.add)
                    mn = fin.tile([P, DH], BF16, tag="mn")
                    nc.vector.tensor_scalar_min(mn[:], v[:], 0.0)
                    em = fin.tile([P, DH], BF16, tag="em")
                    nc.scalar.activation(em[:], mn[:], AF.Exp)
                    o = fin.tile([P, DH], BF16, tag="ob")
                    nc.vector.scalar_tensor_tensor(o[:], v[:], 0.0, em[:],
                                                   op0=AL.max, op1=AL.add)
                    of = fin.tile([P, DH], F32, tag="of")
                    nc.vector.scalar_tensor_tensor(of[:], o[:], -1.0,
                                                   res_sb[:, b],
                                                   op0=AL.add, op1=AL.add)
                    nc.sync.dma_start(outp[b * P:(b + 1) * P, :], of[:])
                    t0 += NBLK
    nc.compile()
    _NC_CACHE[key] = nc
    return nc


# ----------------------------------------------------------------------------
# Host-side preparation
# ----------------------------------------------------------------------------

def wrap_idx(lst: np.ndarray) -> np.ndarray:
    """list index i -> sbuf [16-wrap x 8 replication]: [p, col] = lst[col*16 + p%16]."""
    n = len(lst)
    assert n % 16 == 0
    lay = lst.reshape(n // 16, 16).T.copy()
    return np.tile(lay, (8, 1)).astype(np.int16)


def prep(cfg: Cfg, x, edge_index, ln_g, ln_b, W1, b1, W2, b2, Wr, br, Wg,
         att_src, att_dst, bg):
    """Host-side: sharding, slot layout, idx arrays, weight prep (all static
    except the raw x reformat; activations never touched)."""
    N = cfg.n_nodes
    R = cfg.rows_per_core
    NB = cfg.n_batches
    NCORE = cfg.n_cores
    TR = cfg.table_rows
    H, C = cfg.H, cfg.C

    x = np.asarray(x, np.float32)
    ln_g = np.asarray(ln_g, np.float32)
    ln_b = np.asarray(ln_b, np.float32)
    W1 = np.asarray(W1, np.float32)
    b1 = np.asarray(b1, np.float32)
    W2 = np.asarray(W2, np.float32)
    b2 = np.asarray(b2, np.float32)
    Wr = np.asarray(Wr, np.float32)
    br = np.asarray(br, np.float32)
    Wg = np.asarray(Wg, np.float32)
    att_src = np.asarray(att_src, np.float32)
    att_dst = np.asarray(att_dst, np.float32)
    bg = np.asarray(bg, np.float32)

    src = np.asarray(edge_index[0], np.int64)
    dst = np.asarray(edge_index[1], np.int64)
    loops = np.arange(N, dtype=np.int64)
    src = np.concatenate([src, loops])
    dst = np.concatenate([dst, loops])

    deg = np.bincount(dst, minlength=N)  # in-degree incl self loop

    # ---- node -> core assignment: degree-sorted blocks, round-robin
    order0 = np.argsort(deg, kind="stable")
    padded = np.full(TR, -1, np.int64)
    padded[:N] = order0
    blocks = padded.reshape(TR // P, P)
    core_nodes = [[] for _ in range(NCORE)]
    for j in range(blocks.shape[0]):
        core_nodes[j % NCORE].append(blocks[j])
    core_nodes = [np.concatenate(bl) for bl in core_nodes]  # each [R], -1 dummies
    pos = np.full(N, -1, np.int64)
    for c in range(NCORE):
        ids = core_nodes[c]
        msk = ids >= 0
        pos[ids[msk]] = c * R + np.nonzero(msk)[0]

    spos = pos[src]
    dpos = pos[dst]
    core = dpos // R
    loc = dpos % R
    b_of = loc // P
    dc_of = loc % P
    isB = (spos >= cfg.split).astype(np.int64)

    # per (core, batch, part) counts -> shared block counts
    cnt = np.zeros((NCORE, NB, 2), np.int64)
    np.add.at(cnt, (core, b_of, isB), 1)
    cmax = cnt.max(axis=0)  # [NB, 2]
    nblkA = ((cmax[:, 0] + P - 1) // P).astype(np.int64)
    nblkB = ((cmax[:, 1] + P - 1) // P).astype(np.int64)
    NBT = int(nblkA.sum() + nblkB.sum())

    # slot base per (batch, part): global block offset
    tstart = np.zeros(NB, np.int64)
    tstart[1:] = np.cumsum(nblkA + nblkB)[:-1]
    baseA = tstart * P
    baseB = (tstart + nblkA) * P

    # within-group slot index (grouped by core, batch, part; sorted by spos)
    gkey = ((core * NB + b_of) * 2 + isB)
    order = np.lexsort((spos, gkey))
    gk_s = gkey[order]
    grp_start = np.r_[0, np.nonzero(np.diff(gk_s))[0] + 1]
    sizes = np.diff(np.r_[grp_start, len(gk_s)])
    within = np.arange(len(gk_s)) - np.repeat(grp_start, sizes)
    ks = np.empty(len(gk_s), np.int64)
    ks[order] = within

    slot = np.where(isB == 0, baseA[b_of], baseB[b_of]) + ks

    idxval = np.where(isB == 0, spos, spos - cfg.split).astype(np.int16)

    idx_w, dcol_t, srcg_t, dstloc_t = [], [], [], []
    for c in range(NCORE):
        m = core == c
        sl = slot[m]
        il = np.zeros(NBT * P, np.int16)
        il[sl] = idxval[m]
        sg = np.full(NBT * P, -1, np.int64)
        sg[sl] = spos[m]
        dl = np.full(NBT * P, -1, np.int64)
        dl[sl] = loc[m]
        dca = np.zeros(NBT * P, np.int64)
        dca[sl] = dc_of[m]
        idx_w.append(wrap_idx(il))
        dcol_t.append(dca.reshape(NBT, P).T.astype(NPBF16))
        srcg_t.append(sg.reshape(NBT, P).T.copy())
        dstloc_t.append(dl.reshape(NBT, P).T.copy())

    # ---- phase-1 inputs
    W1p = W1 * ln_g[:, None]
    W1pad = np.zeros((cfg.d_in_pad, cfg.d_hid), np.float32)
    W1pad[:cfg.d_in] = W1p
    w1s = np.zeros((8, cfg.d_hid), np.float32)
    w1s[0] = -W1pad.sum(axis=0)
    cvec_flat = b1 + ln_b @ W1
    cvec = cvec_flat.reshape(cfg.d_hid // P, P).T.astype(np.float32).copy()
    b2t = b2.reshape(cfg.d_out // P, P).T.astype(np.float32).copy()
    onep = np.zeros((8, P), np.float32)
    onep[0] = 1.0
    ones1 = np.ones((P, 1), np.float32)

    att_src_e = np.zeros((cfg.d_head, H), np.float32)
    att_dst_e = np.zeros((cfg.d_head, H), np.float32)
    for h in range(H):
        att_src_e[h * C:(h + 1) * C, h] = att_src[h]
        att_dst_e[h * C:(h + 1) * C, h] = att_dst[h]
    Wgp = np.concatenate([Wg, Wg @ att_src_e], axis=1).astype(np.float32)
    Wrp = np.concatenate([Wr + 0.0, Wg @ att_dst_e], axis=1).astype(np.float32)

    xts = []
    for c in range(NCORE):
        ids = core_nodes[c]
        xs = np.zeros((R, cfg.d_in), np.float32)
        msk = ids >= 0
        xs[msk] = x[ids[msk]]
        xt = np.zeros((cfg.d_in_pad, R), np.float32)
        xt[:cfg.d_in] = xs.T
        xts.append(xt)

    W3 = cfg.d_head + cfg.H
    brpad_t = np.zeros((P, W3), np.float32)
    brpad_t[:, :cfg.d_head] = np.tile(br, (P, 1))
    bgb = np.tile(bg, (P, 1)).astype(np.float32)
    iota = np.tile(np.arange(P, dtype=np.float32), (P, 1)).astype(NPBF16)

    meta = dict(core_nodes=core_nodes, pos=pos,
                nblkA=[int(v) for v in nblkA], nblkB=[int(v) for v in nblkB],
                NBT=NBT, idx_w=idx_w, dcol_t=dcol_t,
                srcg_t=srcg_t, dstloc_t=dstloc_t, bgb=bgb, iota=iota)
    p1_shared = dict(W1p=W1pad, W2=W2, Wgp=Wgp, Wrp=Wrp, w1s=w1s, onep=onep,
                     ones1=ones1, cvec=cvec, b2v=b2t, brpad=brpad_t)
    p1_maps = [dict(xT=xts[c], **p1_shared) for c in range(NCORE)]
    return p1_maps, meta


def make_p2_maps(cfg: Cfg, meta, r1):
    """Between-phase host step: concat table shards, build per-slot streams."""
    NCORE = cfg.n_cores
    gtab_full = np.concatenate([r1[c]["gtab"] for c in range(NCORE)], axis=0)
    asrc_glob = np.concatenate([r1[c]["asrc"] for c in range(NCORE)], axis=0)
    p2_maps = []
    for c in range(NCORE):
        sg = meta["srcg_t"][c]
        dl = meta["dstloc_t"][c]
        aS = asrc_glob[np.maximum(sg, 0)]
        aS[sg < 0] = -4e4
        ad_core = r1[c]["adst"]
        aD = ad_core[np.maximum(dl, 0)]
        aD[dl < 0] = 0.0
        p2_maps.append(dict(
            gtab=gtab_full, idx=meta["idx_w"][c],
            asrcS=aS.astype(NPBF16), adstS=aD.astype(NPBF16),
            dcol=meta["dcol_t"][c], iot=meta["iota"],
            resi=r1[c]["res"], bgb=meta["bgb"],
        ))
    return p2_maps


def kernel(**inputs) -> np.ndarray:
    cfg = CFG
    N = cfg.n_nodes
    NCORE = cfg.n_cores

    p1_maps, meta = prep(cfg, **inputs)

    nc1 = build_phase1(cfg)
    r1 = run_bass_kernel_spmd(nc1, p1_maps, core_ids=list(range(NCORE))).results

    nc2 = build_phase2(cfg, meta["nblkA"], meta["nblkB"])
    p2_maps = make_p2_maps(cfg, meta, r1)
    r2 = run_bass_kernel_spmd(nc2, p2_maps, core_ids=list(range(NCORE))).results

    out = np.zeros((N, cfg.d_head), np.float32)
    for c in range(NCORE):
        ids = meta["core_nodes"][c]
        msk = ids >= 0
        out[ids[msk]] = r2[c]["outp"][msk]
    return out


# revision 11
# speedup vs baseline: 1.1295x; 1.1295x over previous
"""Trainium2 Bass kernel for nn_BaselineGAT (LayerNorm + MLP + GATConv).

Strategy (8 NeuronCores, SPMD, host-mediated phase boundary):
  Phase 1 (per core, nodes sharded 6272/core, degree-bucketed order):
    LayerNorm folded into the first matmul (stats via ones-matmul + Square),
    MLP 1488->1024->512 in float32r (full-rate PE), then row-major heads:
    g table rows (bf16), res rows (bf16), a_src/a_dst side tables (f32).
  Host: concat g-table shards -> full bf16 table [50176, 256] (512B rows);
    build per-edge-slot a_src/a_dst streams by static slot indexing.
  Phase 2 (per core, edges sharded by dst, ragged 128-edge blocks per
    128-dst batch, src table split at 32768 for int16 gather indices):
    dma_gather of g rows (one edge per partition), w = exp(leaky_relu(
    a_src+a_dst)) on DVE/ACT, msg = [g*w | w] bf16, one-hot S = (dcol==iota),
    PE scatter-reduce out[dst,f] = S^T @ msg accumulated over blocks in PSUM,
    then normalize by the denominator column, elu, + res -> output rows.
    Empty slots use idx 0 with a_src = -4e4 so w == 0 exactly.
"""

import sys

sys.path.insert(0, "/opt/trn_rl_repo")

from dataclasses import dataclass

import numpy as np
import ml_dtypes

import concourse.bass as bass  # noqa: F401
import concourse.mybir as mybir
import concourse.tile as tile
from concourse import bacc
from concourse.bass_utils import run_bass_kernel_spmd
from concourse.library_config import mlp as mlp_lib

P = 128
F32 = mybir.dt.float32
F32R = mybir.dt.float32r
BF16 = mybir.dt.bfloat16
I16 = mybir.dt.int16
AL = mybir.AluOpType
AF = mybir.ActivationFunctionType
NPBF16 = ml_dtypes.bfloat16


@dataclass
class Cfg:
    n_nodes: int = 50000
    n_edges: int = 800000
    d_in: int = 1488
    d_hid: int = 1024
    d_out: int = 512
    C: int = 32
    H: int = 8
    n_cores: int = 8
    node_chunk: int = 384  # phase-1 nodes per chunk
    split_cap: int = 32768  # max rows addressable by int16 gather idx
    gmax: int = 8  # max 128-edge blocks (1024 idxs) per dma_gather

    @property
    def d_head(self):  # H*C
        return self.C * self.H

    @property
    def d_in_pad(self):
        return ((self.d_in + P - 1) // P) * P

    @property
    def rows_per_core(self):
        nb = (self.n_nodes + P - 1) // P
        nb = ((nb + self.n_cores - 1) // self.n_cores) * self.n_cores
        return nb // self.n_cores * P

    @property
    def n_batches(self):
        return self.rows_per_core // P

    @property
    def table_rows(self):
        return self.rows_per_core * self.n_cores

    @property
    def split(self):
        # pass-A half of the table (int16 index limit)
        return min(self.split_cap, self.table_rows)


CFG = Cfg()

_NC_CACHE = {}


# ----------------------------------------------------------------------------
# Phase 1: LayerNorm + MLP + heads
# ----------------------------------------------------------------------------

def build_phase1(cfg: Cfg, repeat: int = 1):
    # repeat>1 builds are timing-only variants (same outputs rewritten)
    key = ("p1", cfg.n_nodes, cfg.node_chunk, repeat)
    if key in _NC_CACHE:
        return _NC_CACHE[key]
    nc = bacc.Bacc("TRN2", target_bir_lowering=False)
    R = cfg.rows_per_core
    NB = cfg.n_batches
    KT1 = cfg.d_in_pad // P         # k-tiles layer 1
    KT2 = cfg.d_hid // P            # k-tiles layer 2
    KT3 = cfg.d_out // P            # k-tiles layer 3
    MT1 = cfg.d_hid // P            # m-tiles layer 1
    MT2 = cfg.d_out // P            # m-tiles layer 2
    NCK = cfg.node_chunk
    chunk_sizes = [NCK] * (R // NCK)
    if R % NCK:
        assert R % NCK % P == 0
        chunk_sizes.append(R % NCK)
    W3 = cfg.d_head + cfg.H         # 264
    DH = cfg.d_head

    xT = nc.dram_tensor("xT", [cfg.d_in_pad, R], F32R, kind="ExternalInput")
    W1p = nc.dram_tensor("W1p", [cfg.d_in_pad, cfg.d_hid], F32R, kind="ExternalInput")
    W2 = nc.dram_tensor("W2", [cfg.d_hid, cfg.d_out], F32R, kind="ExternalInput")
    Wgp = nc.dram_tensor("Wgp", [cfg.d_out, W3], F32R, kind="ExternalInput")
    Wrp = nc.dram_tensor("Wrp", [cfg.d_out, W3], F32R, kind="ExternalInput")
    w1s = nc.dram_tensor("w1s", [8, cfg.d_hid], F32R, kind="ExternalInput")  # row0 = -colsum(W1p)
    onep = nc.dram_tensor("onep", [8, P], F32R, kind="ExternalInput")        # row0 = ones
    ones1 = nc.dram_tensor("ones1", [P, 1], F32R, kind="ExternalInput")
    cvec = nc.dram_tensor("cvec", [P, MT1], F32, kind="ExternalInput")      # b1 + ln_b@W1
    b2v = nc.dram_tensor("b2v", [P, MT2], F32, kind="ExternalInput")
    brpad = nc.dram_tensor("brpad", [P, W3], F32, kind="ExternalInput")

    gtab = nc.dram_tensor("gtab", [R, DH], BF16, kind="ExternalOutput")
    res = nc.dram_tensor("res", [R, DH], BF16, kind="ExternalOutput")
    asrc = nc.dram_tensor("asrc", [R, cfg.H], F32, kind="ExternalOutput")
    adst = nc.dram_tensor("adst", [R, cfg.H], F32, kind="ExternalOutput")

    inv_din = 1.0 / cfg.d_in

    with tile.TileContext(nc) as tc:
        with (
            tc.tile_pool(name="wpool", bufs=1) as wp,
            tc.tile_pool(name="xpool", bufs=2) as xp,
            tc.tile_pool(name="sqpool", bufs=2) as sqp,
            tc.tile_pool(name="hpool", bufs=2) as hp,
            tc.tile_pool(name="epool", bufs=3) as ep,
            tc.tile_pool(name="stat", bufs=1) as stp,
            tc.tile_pool(name="ps_y", bufs=2, space="PSUM") as ps_y,
            tc.tile_pool(name="ps_s", bufs=1, space="PSUM") as ps_s,
            tc.tile_pool(name="ps_o", bufs=3, space="PSUM") as ps_o,
        ):
            w1_sb = wp.tile([P, KT1, cfg.d_hid], F32R)
            nc.sync.dma_start(w1_sb[:], W1p.rearrange("(kt p) m -> p kt m", p=P))
            w2_sb = wp.tile([P, KT2, cfg.d_out], F32R)
            nc.sync.dma_start(w2_sb[:], W2.rearrange("(kt p) m -> p kt m", p=P))
            wg_sb = wp.tile([P, KT3, W3], F32R)
            nc.sync.dma_start(wg_sb[:], Wgp.rearrange("(kt p) m -> p kt m", p=P))
            wr_sb = wp.tile([P, KT3, W3], F32R)
            nc.sync.dma_start(wr_sb[:], Wrp.rearrange("(kt p) m -> p kt m", p=P))
            w1s_sb = wp.tile([8, cfg.d_hid], F32R)
            nc.sync.dma_start(w1s_sb[:], w1s[:])
            onep_sb = wp.tile([8, P], F32R)
            nc.sync.dma_start(onep_sb[:], onep[:])
            ones1_sb = wp.tile([P, 1], F32R)
            nc.sync.dma_start(ones1_sb[:], ones1[:])
            cvec_sb = wp.tile([P, MT1], F32)
            nc.sync.dma_start(cvec_sb[:], cvec[:])
            b2_sb = wp.tile([P, MT2], F32)
            nc.sync.dma_start(b2_sb[:], b2v[:])
            brp_sb = wp.tile([P, W3], F32)
            nc.sync.dma_start(brp_sb[:], brpad[:])
            asrc_acc = wp.tile([P, NB, cfg.H], F32)
            adst_acc = wp.tile([P, NB, cfg.H], F32)

            for _rep in range(repeat):
                ns = 0
                for NC in chunk_sizes:
                    # ---- load xT chunk [P, KT1, NC]
                    xt = xp.tile([P, KT1, NC], F32R, tag="xt")
                    nc.sync.dma_start(
                        xt[:],
                        xT.rearrange("(kt p) n -> p kt n", p=P)[:, :, ns:ns + NC]
                    )
                    # ---- stats: S1 = ones^T @ x ; S2 = ones^T @ x^2
                    s1_ps = ps_s.tile([1, NC], F32, tag="s1")
                    s2_ps = ps_s.tile([1, NC], F32, tag="s2")
                    for kt in range(KT1):
                        nc.tensor.matmul(s1_ps[:], ones1_sb[:], xt[:, kt],
                                         start=(kt == 0), stop=(kt == KT1 - 1))
                    for kt in range(KT1):
                        xsq = sqp.tile([P, NC], F32R, tag="xsq")
                        nc.scalar.activation(xsq[:], xt[:, kt], AF.Square)
                        nc.tensor.matmul(s2_ps[:], ones1_sb[:], xsq[:],
                                         start=(kt == 0), stop=(kt == KT1 - 1))
                    # ---- finalize stats: mu, rstd (on [1, NC])
                    mu_f = stp.tile([8, NC], F32, tag="muf")
                    nc.vector.memset(mu_f[:], 0.0)
                    nc.vector.tensor_scalar_mul(mu_f[0:1, :], s1_ps[:], inv_din)
                    mu_pad = stp.tile([8, NC], F32R, tag="mu")
                    nc.scalar.activation(mu_pad[:], mu_f[:], AF.Identity)
                    musq = stp.tile([1, NC], F32, tag="musq")
                    nc.vector.tensor_tensor(musq[:], mu_f[0:1, :], mu_f[0:1, :],
                                            op=AL.mult)
                    var = stp.tile([1, NC], F32, tag="var")
                    # var = S2/din - mu^2 + eps
                    nc.vector.tensor_scalar(var[:], s2_ps[:], inv_din, None,
                                            op0=AL.mult)
                    nc.vector.tensor_tensor(var[:], var[:], musq[:],
                                            op=AL.subtract)
                    nc.vector.tensor_scalar_add(var[:], var[:], 1e-5)
                    sd = stp.tile([8, NC], F32, tag="sd")
                    nc.scalar.activation(sd[0:1, :], var[:], AF.Sqrt)
                    rstd_f = stp.tile([8, NC], F32, tag="rstdf")
                    nc.vector.memset(rstd_f[:], 0.0)
                    nc.vector.reciprocal(rstd_f[0:1, :], sd[0:1, :])
                    rstd = stp.tile([8, NC], F32R, tag="rstd")
                    nc.scalar.activation(rstd[:], rstd_f[:], AF.Identity)
                    # broadcast rstd to [P, NC] via K=8 matmul
                    rb_ps = ps_s.tile([P, NC], F32, tag="rb")
                    nc.tensor.matmul(rb_ps[:], onep_sb[:], rstd[:],
                                     start=True, stop=True)
                    rstd_b = stp.tile([P, NC], F32, tag="rstdb")
                    nc.vector.tensor_copy(rstd_b[:], rb_ps[:])

                    # ---- layer 1: y = W1p^T x - w1sum (x) mu; h = relu(y*rstd+c)
                    h_sb = hp.tile([P, MT1, NC], F32R, tag="h")
                    for mt in range(MT1):
                        y_ps = ps_y.tile([P, NC], F32, tag="y")
                        for kt in range(KT1):
                            nc.tensor.matmul(
                                y_ps[:], w1_sb[:, kt, mt * P:(mt + 1) * P],
                                xt[:, kt], start=(kt == 0), stop=False)
                        nc.tensor.matmul(
                            y_ps[:], w1s_sb[:, mt * P:(mt + 1) * P], mu_pad[:],
                            start=False, stop=True)
                        tmp = ep.tile([P, NC], F32, tag="l1t")
                        nc.vector.tensor_tensor(tmp[:], y_ps[:], rstd_b[:],
                                                op=AL.mult)
                        nc.scalar.activation(h_sb[:, mt], tmp[:], AF.Relu,
                                             bias=cvec_sb[:, mt:mt + 1])

                    # ---- layer 2: h2 = W2^T h + b2
                    h2_sb = hp.tile([P, MT2, NC], F32R, tag="h2")
                    for mt in range(MT2):
                        y2_ps = ps_y.tile([P, NC], F32, tag="y")
                        for kt in range(KT2):
                            nc.tensor.matmul(
                                y2_ps[:], w2_sb[:, kt, mt * P:(mt + 1) * P],
                                h_sb[:, kt], start=(kt == 0),
                                stop=(kt == KT2 - 1))
                        nc.scalar.activation(h2_sb[:, mt], y2_ps[:], AF.Identity,
                                             bias=b2_sb[:, mt:mt + 1])

                    # ---- layer 3 (row-major): per 128-node subtile
                    for nt in range(NC // P):
                        g_ps = ps_o.tile([P, W3], F32, tag="ops")
                        r_ps = ps_o.tile([P, W3], F32, tag="ops")
                        for kt in range(KT3):
                            nc.tensor.matmul(
                                g_ps[:], h2_sb[:, kt, nt * P:(nt + 1) * P],
                                wg_sb[:, kt], start=(kt == 0),
                                stop=(kt == KT3 - 1))
                        for kt in range(KT3):
                            nc.tensor.matmul(
                                r_ps[:], h2_sb[:, kt, nt * P:(nt + 1) * P],
                                wr_sb[:, kt], start=(kt == 0),
                                stop=(kt == KT3 - 1))
                        r0 = ns + nt * P
                        bidx = r0 // P
                        gt = ep.tile([P, DH], BF16, tag="gt")
                        nc.scalar.activation(gt[:], g_ps[:, :DH], AF.Identity)
                        nc.vector.tensor_copy(asrc_acc[:, bidx], g_ps[:, DH:W3])
                        rt = ep.tile([P, DH], BF16, tag="rt")
                        nc.vector.tensor_tensor(rt[:], r_ps[:, :DH],
                                                brp_sb[:, :DH], op=AL.add)
                        nc.vector.tensor_copy(adst_acc[:, bidx], r_ps[:, DH:W3])
                        nc.sync.dma_start(gtab[r0:r0 + P, :], gt[:])
                        nc.sync.dma_start(res[r0:r0 + P, :], rt[:])
                    ns += NC
            nc.sync.dma_start(asrc.rearrange("(b p) w -> p b w", p=P), asrc_acc[:])
            nc.sync.dma_start(adst.rearrange("(b p) w -> p b w", p=P), adst_acc[:])
    nc.compile()
    _NC_CACHE[key] = nc
    return nc


# ----------------------------------------------------------------------------
# Phase 2: gather + edge softmax + one-hot scatter matmul + finalize
# ----------------------------------------------------------------------------

def build_phase2(cfg: Cfg, nblkA: list, nblkB: list, repeat: int = 1):
    """nblkA/nblkB: per-batch 128-edge block counts for the A half
    (src row < split) and B half of the table; shared across cores."""
    key = ("p2", cfg.n_nodes, tuple(nblkA), tuple(nblkB), repeat)
    if key in _NC_CACHE:
        return _NC_CACHE[key]
    nc = bacc.Bacc("TRN2", target_bir_lowering=False)
    R = cfg.rows_per_core
    NB = cfg.n_batches
    DH = cfg.d_head
    H = cfg.H
    C = cfg.C
    W = DH + H  # msg row: [g*w | w]
    assert len(nblkA) == NB and len(nblkB) == NB
    NBT = int(sum(nblkA) + sum(nblkB))
    NBLKMAX = int(max(a + b for a, b in zip(nblkA, nblkB)))
    GMAX = cfg.gmax

    gtab = nc.dram_tensor("gtab", [cfg.table_rows, DH], BF16, kind="ExternalInput")
    idx = nc.dram_tensor("idx", [P, 8 * NBT], I16, kind="ExternalInput")
    asrcS = nc.dram_tensor("asrcS", [P, NBT, H], BF16, kind="ExternalInput")
    adstS = nc.dram_tensor("adstS", [P, NBT, H], BF16, kind="ExternalInput")
    Sfull = nc.dram_tensor("Sfull", [P, NBT, P], BF16, kind="ExternalInput")
    resi = nc.dram_tensor("resi", [R, DH], BF16, kind="ExternalInput")
    bgb = nc.dram_tensor("bgb", [P, DH], F32, kind="ExternalInput")
    outp = nc.dram_tensor("outp", [R, DH], F32, kind="ExternalOutput")

    tabA = gtab[:cfg.split, :]
    tabB = gtab[cfg.split:, :]

    with tile.TileContext(nc) as tc:
        with (
            tc.tile_pool(name="const", bufs=1) as cp,
            tc.tile_pool(name="gpool", bufs=2) as gp,
            tc.tile_pool(name="spool", bufs=2) as sp,
            tc.tile_pool(name="mpool", bufs=2) as mp,
            tc.tile_pool(name="wk", bufs=3) as wk,
            tc.tile_pool(name="fin", bufs=3) as fin,
            tc.tile_pool(name="pso", bufs=4, space="PSUM") as pso,
        ):
            nc.gpsimd.load_library(mlp_lib)
            idx_sb = cp.tile([P, 8 * NBT], I16)
            nc.sync.dma_start(idx_sb[:], idx[:])
            asrc_sb = cp.tile([P, NBT, H], BF16)
            nc.sync.dma_start(asrc_sb[:], asrcS[:])
            adst_sb = cp.tile([P, NBT, H], BF16)
            nc.sync.dma_start(adst_sb[:], adstS[:])
            res_sb = cp.tile([P, NB, DH], BF16)
            nc.sync.dma_start(res_sb[:], resi.rearrange("(b p) w -> p b w", p=P))
            bg_sb = cp.tile([P, DH], F32)
            nc.sync.dma_start(bg_sb[:], bgb[:])

            for _rep in range(repeat):
                t0 = 0
                for b in range(NB):
                    na, nbk = int(nblkA[b]), int(nblkB[b])
                    NBLK = na + nbk
                    assert NBLK >= 1
                    G = gp.tile([P, NBLKMAX, DH], BF16, tag="g")
                    # gathers: A blocks then B blocks (<=GMAX blocks per call)
                    for part, cnt, base, tab in ((0, na, 0, tabA),
                                                 (1, nbk, na, tabB)):
                        for j0 in range(0, cnt, GMAX):
                            kk = min(GMAX, cnt - j0)
                            ni = P * kk
                            tcol = 8 * (t0 + base + j0)
                            nc.gpsimd.dma_gather(
                                G[:, base + j0:base + j0 + kk, :], tab,
                                idx_sb[:, tcol:tcol + 8 * kk], ni, ni, DH,
                            )
                    # one-hot S (host precomputed, static): stream per batch
                    S = sp.tile([P, NBLKMAX, P], BF16, tag="s")
                    nc.sync.dma_start(S[:, :NBLK], Sfull[:, t0:t0 + NBLK, :])
                    # w = exp(leaky_relu(a_src + a_dst))
                    ww = wk.tile([P, NBLKMAX, H], BF16, tag="ww")
                    nc.vector.tensor_tensor(ww[:, :NBLK],
                                            asrc_sb[:, t0:t0 + NBLK],
                                            adst_sb[:, t0:t0 + NBLK], op=AL.add)
                    nc.vector.scalar_tensor_tensor(
                        ww[:, :NBLK], ww[:, :NBLK], 0.2, ww[:, :NBLK],
                        op0=AL.mult, op1=AL.max)
                    wb = wk.tile([P, NBLKMAX, H], BF16, tag="wb")
                    nc.scalar.activation(wb[:, :NBLK], ww[:, :NBLK], AF.Exp)
                    # msg = [g * w | w]  (c-major feature order: inner dim = h,
                    # so the w broadcast keeps stride-1 inner reads -> DVE 2x)
                    M = mp.tile([P, NBLKMAX, W], BF16, tag="m")
                    nc.vector.tensor_tensor(
                        M[:, :NBLK, :DH].rearrange("p k (c h) -> p k c h", h=H),
                        G[:, :NBLK].rearrange("p k (c h) -> p k c h", h=H),
                        wb[:, :NBLK].unsqueeze(2).to_broadcast([P, NBLK, C, H]),
                        op=AL.mult)
                    nc.vector.tensor_copy(M[:, :NBLK, DH:W], wb[:, :NBLK])
                    # scatter-reduce: out[d, f] = sum_e S[e,d] * msg[e,f]
                    o_ps = pso.tile([P, W], F32, tag="o")
                    for j in range(NBLK):
                        nc.tensor.matmul(o_ps[:], S[:, j], M[:, j],
                                         start=(j == 0), stop=(j == NBLK - 1))
                    # normalize, elu, + res
                    rec = fin.tile([P, H], F32, tag="rec")
                    nc.vector.tensor_scalar_add(rec[:], o_ps[:, DH:W], 1e-16)
                    nc.vector.reciprocal(rec[:], rec[:])
                    z = fin.tile([P, DH], F32, tag="z")
                    nc.vector.tensor_tensor(
                        z[:].rearrange("p (c h) -> p c h", h=H),
                        o_ps[:, :DH].rearrange("p (c h) -> p c h", h=H),
                        rec[:].unsqueeze(1).to_broadcast([P, C, H]),
                        op=AL.mult)
                    v = fin.tile([P, DH], BF16, tag="v")
                    nc.vector.tensor_tensor(v[:], z[:], bg_sb[:], op=AL.add)
                    mn = fin.tile([P, DH], BF16, tag="mn")
                    nc.vector.tensor_scalar_min(mn[:], v[:], 0.0)
                    em = fin.tile([P, DH], BF16, tag="em")
                    nc.scalar.activation(em[:], mn[:], AF.Exp)
                    o = fin.tile([P, DH], BF16, tag="ob")
                    nc.vector.scalar_tensor_tensor(o[:], v[:], 0.0, em[:],
                                                   op0=AL.max, op1=AL.add)
                    of = fin.tile([P, DH], F32, tag="of")
                    nc.vector.scalar_tensor_tensor(of[:], o[:], -1.0,
                                                   res_sb[:, b],
                                                   op0=AL.add, op1=AL.add)
                    nc.sync.dma_start(outp[b * P:(b + 1) * P, :], of[:])
                    t0 += NBLK
    nc.compile()
    _NC_CACHE[key] = nc
    return nc


# ----------------------------------------------------------------------------
# Host-side preparation
# ----------------------------------------------------------------------------

def wrap_idx(lst: np.ndarray) -> np.ndarray:
    """list index i -> sbuf [16-wrap x 8 replication]: [p, col] = lst[col*16 + p%16]."""
    n = len(lst)
    assert n % 16 == 0
    lay = lst.reshape(n // 16, 16).T.copy()
    return np.tile(lay, (8, 1)).astype(np.int16)


def prep(cfg: Cfg, x, edge_index, ln_g, ln_b, W1, b1, W2, b2, Wr, br, Wg,
         att_src, att_dst, bg):
    """Host-side: sharding, slot layout, idx arrays, weight prep (all static
    except the raw x reformat; activations never touched)."""
    N = cfg.n_nodes
    R = cfg.rows_per_core
    NB = cfg.n_batches
    NCORE = cfg.n_cores
    TR = cfg.table_rows
    H, C = cfg.H, cfg.C

    x = np.asarray(x, np.float32)
    ln_g = np.asarray(ln_g, np.float32)
    ln_b = np.asarray(ln_b, np.float32)
    W1 = np.asarray(W1, np.float32)
    b1 = np.asarray(b1, np.float32)
    W2 = np.asarray(W2, np.float32)
    b2 = np.asarray(b2, np.float32)
    Wr = np.asarray(Wr, np.float32)
    br = np.asarray(br, np.float32)
    Wg = np.asarray(Wg, np.float32)
    att_src = np.asarray(att_src, np.float32)
    att_dst = np.asarray(att_dst, np.float32)
    bg = np.asarray(bg, np.float32)

    src = np.asarray(edge_index[0], np.int64)
    dst = np.asarray(edge_index[1], np.int64)
    loops = np.arange(N, dtype=np.int64)
    src = np.concatenate([src, loops])
    dst = np.concatenate([dst, loops])

    deg = np.bincount(dst, minlength=N)  # in-degree incl self loop

    # ---- node -> core assignment: degree-sorted blocks, round-robin
    order0 = np.argsort(deg, kind="stable")
    padded = np.full(TR, -1, np.int64)
    padded[:N] = order0
    blocks = padded.reshape(TR // P, P)
    core_nodes = [[] for _ in range(NCORE)]
    for j in range(blocks.shape[0]):
        core_nodes[j % NCORE].append(blocks[j])
    core_nodes = [np.concatenate(bl) for bl in core_nodes]  # each [R], -1 dummies
    pos = np.full(N, -1, np.int64)
    for c in range(NCORE):
        ids = core_nodes[c]
        msk = ids >= 0
        pos[ids[msk]] = c * R + np.nonzero(msk)[0]

    spos = pos[src]
    dpos = pos[dst]
    core = dpos // R
    loc = dpos % R
    b_of = loc // P
    dc_of = loc % P
    isB = (spos >= cfg.split).astype(np.int64)

    # per (core, batch, part) counts -> shared block counts
    cnt = np.zeros((NCORE, NB, 2), np.int64)
    np.add.at(cnt, (core, b_of, isB), 1)
    cmax = cnt.max(axis=0)  # [NB, 2]
    nblkA = ((cmax[:, 0] + P - 1) // P).astype(np.int64)
    nblkB = ((cmax[:, 1] + P - 1) // P).astype(np.int64)
    NBT = int(nblkA.sum() + nblkB.sum())

    # slot base per (batch, part): global block offset
    tstart = np.zeros(NB, np.int64)
    tstart[1:] = np.cumsum(nblkA + nblkB)[:-1]
    baseA = tstart * P
    baseB = (tstart + nblkA) * P

    # within-group slot index (grouped by core, batch, part; sorted by spos)
    gkey = ((core * NB + b_of) * 2 + isB)
    order = np.lexsort((spos, gkey))
    gk_s = gkey[order]
    grp_start = np.r_[0, np.nonzero(np.diff(gk_s))[0] + 1]
    sizes = np.diff(np.r_[grp_start, len(gk_s)])
    within = np.arange(len(gk_s)) - np.repeat(grp_start, sizes)
    ks = np.empty(len(gk_s), np.int64)
    ks[order] = within

    slot = np.where(isB == 0, baseA[b_of], baseB[b_of]) + ks

    idxval = np.where(isB == 0, spos, spos - cfg.split).astype(np.int16)

    darange = np.arange(P, dtype=np.int64)
    idx_w, S_t, srcg_t, dstloc_t = [], [], [], []
    for c in range(NCORE):
        m = core == c
        sl = slot[m]
        il = np.zeros(NBT * P, np.int16)
        il[sl] = idxval[m]
        sg = np.full(NBT * P, -1, np.int64)
        sg[sl] = spos[m]
        dl = np.full(NBT * P, -1, np.int64)
        dl[sl] = loc[m]
        dca = np.full(NBT * P, -1, np.int64)
        dca[sl] = dc_of[m]
        idx_w.append(wrap_idx(il))
        S = (dca[:, None] == darange[None, :])
        S_t.append(S.reshape(NBT, P, P).transpose(1, 0, 2)
                   .astype(NPBF16).copy())
        srcg_t.append(sg.reshape(NBT, P).T.copy())
        dstloc_t.append(dl.reshape(NBT, P).T.copy())

    # ---- phase-1 inputs
    W1p = W1 * ln_g[:, None]
    W1pad = np.zeros((cfg.d_in_pad, cfg.d_hid), np.float32)
    W1pad[:cfg.d_in] = W1p
    w1s = np.zeros((8, cfg.d_hid), np.float32)
    w1s[0] = -W1pad.sum(axis=0)
    cvec_flat = b1 + ln_b @ W1
    cvec = cvec_flat.reshape(cfg.d_hid // P, P).T.astype(np.float32).copy()
    b2t = b2.reshape(cfg.d_out // P, P).T.astype(np.float32).copy()
    onep = np.zeros((8, P), np.float32)
    onep[0] = 1.0
    ones1 = np.ones((P, 1), np.float32)

    att_src_e = np.zeros((cfg.d_head, H), np.float32)
    att_dst_e = np.zeros((cfg.d_head, H), np.float32)
    for h in range(H):
        att_src_e[h * C:(h + 1) * C, h] = att_src[h]
        att_dst_e[h * C:(h + 1) * C, h] = att_dst[h]
    # c-major feature permutation: new col j=(c,h) <- old col h*C+c
    ar = np.arange(cfg.d_head)
    perm_cm = (ar % H) * C + (ar // H)
    inv_cm = (ar % C) * H + (ar // C)
    Wgp = np.concatenate([Wg[:, perm_cm], Wg @ att_src_e],
                         axis=1).astype(np.float32)
    Wrp = np.concatenate([Wr[:, perm_cm], Wg @ att_dst_e],
                         axis=1).astype(np.float32)

    xts = []
    for c in range(NCORE):
        ids = core_nodes[c]
        xs = np.zeros((R, cfg.d_in), np.float32)
        msk = ids >= 0
        xs[msk] = x[ids[msk]]
        xt = np.zeros((cfg.d_in_pad, R), np.float32)
        xt[:cfg.d_in] = xs.T
        xts.append(xt)

    W3 = cfg.d_head + cfg.H
    brpad_t = np.zeros((P, W3), np.float32)
    brpad_t[:, :cfg.d_head] = np.tile(br[perm_cm], (P, 1))
    bgb = np.tile(bg[perm_cm], (P, 1)).astype(np.float32)

    meta = dict(core_nodes=core_nodes, pos=pos,
                nblkA=[int(v) for v in nblkA], nblkB=[int(v) for v in nblkB],
                NBT=NBT, idx_w=idx_w, S_t=S_t, inv_cm=inv_cm,
                srcg_t=srcg_t, dstloc_t=dstloc_t, bgb=bgb)
    p1_shared = dict(W1p=W1pad, W2=W2, Wgp=Wgp, Wrp=Wrp, w1s=w1s, onep=onep,
                     ones1=ones1, cvec=cvec, b2v=b2t, brpad=brpad_t)
    p1_maps = [dict(xT=xts[c], **p1_shared) for c in range(NCORE)]
    return p1_maps, meta


def make_p2_maps(cfg: Cfg, meta, r1):
    """Between-phase host step: concat table shards, build per-slot streams."""
    NCORE = cfg.n_cores
    gtab_full = np.concatenate([r1[c]["gtab"] for c in range(NCORE)], axis=0)
    asrc_glob = np.concatenate([r1[c]["asrc"] for c in range(NCORE)], axis=0)
    p2_maps = []
    for c in range(NCORE):
        sg = meta["srcg_t"][c]
        dl = meta["dstloc_t"][c]
        aS = asrc_glob[np.maximum(sg, 0)]
        aS[sg < 0] = -4e4
        ad_core = r1[c]["adst"]
        aD = ad_core[np.maximum(dl, 0)]
        aD[dl < 0] = 0.0
        p2_maps.append(dict(
            gtab=gtab_full, idx=meta["idx_w"][c],
            asrcS=aS.astype(NPBF16), adstS=aD.astype(NPBF16),
            Sfull=meta["S_t"][c],
            resi=r1[c]["res"], bgb=meta["bgb"],
        ))
    return p2_maps


def kernel(**inputs) -> np.ndarray:
    cfg = CFG
    N = cfg.n_nodes
    NCORE = cfg.n_cores

    p1_maps, meta = prep(cfg, **inputs)

    nc1 = build_phase1(cfg)
    r1 = run_bass_kernel_spmd(nc1, p1_maps, core_ids=list(range(NCORE))).results

    nc2 = build_phase2(cfg, meta["nblkA"], meta["nblkB"])
    p2_maps = make_p2_maps(cfg, meta, r1)
    r2 = run_bass_kernel_spmd(nc2, p2_maps, core_ids=list(range(NCORE))).results

    out = np.zeros((N, cfg.d_head), np.float32)
    inv = meta["inv_cm"]
    for c in range(NCORE):
        ids = meta["core_nodes"][c]
        msk = ids >= 0
        out[ids[msk]] = r2[c]["outp"][msk][:, inv]
    return out
